# revision 28
# baseline (speedup 1.0000x reference)
# Trainium2 Bass kernel for the LeNet-C3 sparse-connection conv problem.
#
# Math: VALID 2D conv, input [32, 512, 512, 6] f32, dense kernel [5,5,6,16]
# (assembled from the sparse C3 connection tables), + bias -> [32, 508, 508, 16].
#
# Strategy (pure data parallel, 4 images per core x 8 cores):
#   Matmul roles are "swapped" vs the obvious im2col mapping so the PSUM
#   output lands with partition = y (image row), making the output store a
#   contiguous row-block DMA:
#     psum[y, (xo,co)] += xprep[:, y+dy window].T @ W_mov[dy][:, 256]
#   - stationary (lhsT) = transposed image slice [128, M=128]: partitions are
#     the 20-pixel x 6-channel flat input window of a 16-output-pixel "pair
#     group", plus a constant ones row (row 120) that folds the bias in and
#     zero pad to 128 (128 weights enable fast-weight-load, which halves
#     LDWEIGHTS and takes MM issue spacing from 131ns to the 109ns fill
#     floor). The dy shift is a free-dim column offset (no data movement).
#   - moving (rhs) = W_mov[dy] [128, 256]: row 6u+c, col 16xo+co holds
#     Wd[dy, u-xo, c, co] (zero outside 0<=u-xo<5); row 120 of dy=0 = bias.
#   - 5 accumulating matmuls per (y-block, pair), then ScalarE/VectorE
#     (alternating) copy PSUM -> bf16 staging tile laid out exactly like the
#     DRAM output row block; two contiguous ~1MB stores per row block.
#   Engine/queue split: input loads on the scalar HWDGE ring, output stores
#   on the sync HWDGE ring, so a store's wait-for-drains never head-of-line
#   blocks input prefetch (and vice versa). ~120 dummy matmuls at kernel
#   start warm the PE HAM clock gate during the first input DMA.
#   Host side pre-transposes the input into [n, half, 128, 32*264] bf16 so
#   every device DMA is large and per-partition contiguous (a strided-AP
#   rearrange DMA is packet-flood bound: ~11ms/image on the first attempt).
#   Outputs are written bf16 and upcast to f32 on host (rel err ~3.5e-3,
#   budget 2e-2).
#
#   Measured (neuron-profile, core 0 of the real 8-core program): ~301us per
#   core for 4 images; MM stream at 109ns/MM spacing = PE fill floor for
#   this dataflow (163,840 moving columns/image @ 2.4GHz = 68.3us/image).

import numpy as np
import ml_dtypes

BATCH, H, W, CIN, COUT, FS = 32, 512, 512, 6, 16, 5
N_CORES = 8
IMGS_PER_CORE = BATCH // N_CORES  # 4
HO = WO = H - FS + 1  # 508
FLAT = W * CIN  # 3072
FLAT_PAD = 3104  # >= 96*31 + 121
NPAIR = 32          # pair groups of 16 output pixels
KWIN = 120          # 20 x-positions * 6 channels
KONE = 120          # ones row for bias
KDIM = 128          # padded to 128 so LDWEIGHTS gets fast-weight-load
NMOV = 256          # 16 out pixels * 16 out channels
YH = 264            # y extent per half: covers 2 y-blocks + 4 filter taps
XCOLS = NPAIR * YH  # 8448 free columns per (image, half) input tile
HPAIR = NPAIR // 4  # pairs per input-load chunk
OCOLS = WO * COUT   # 8128 valid output columns per row
# (half, block) -> (cb column offset within a pair's 264 cols, M rows).
# The last block re-computes rows 380-383 (overlap with block 2) so every
# store is a full 128 partitions — 124-partition DMAs only get a 4-way
# SDMA-engine split (78 GB/s vs 400).
BLOCKS = {0: ((0, 128), (128, 128)), 1: ((8, 128), (132, 128))}

_CACHE = {}


def _dense_kernel_np(weights3, weights4, weights4_4, weights6):
    """Numpy port of reference._dense_kernel: [5,5,6,16] dense conv kernel."""
    f = weights3.shape[0]
    Wd = np.zeros((f, f, CIN, COUT), dtype=np.float32)
    for i in range(6):
        for m in range(3):
            Wd[:, :, (i + m) % 6, i] = weights3[:, :, m, i]
    for k in range(6):
        for m in range(4):
            Wd[:, :, (k + m) % 6, 6 + k] = weights4[:, :, m, k]
    for k in range(3):
        for m, off in enumerate((0, 1, 3, 4)):
            Wd[:, :, (k + off) % 6, 12 + k] = weights4_4[:, :, m, k]
    Wd[:, :, :, 15] = weights6[:, :, :, 0]
    return Wd


def _build_wmov(Wd, bias1):
    """[KDIM, FS*NMOV]: col dy*256 + 16*xo + co, row 6u+c -> Wd[dy,u-xo,c,co];
    row 120 of the dy=0 slab carries the bias."""
    wm = np.zeros((KDIM, FS, NMOV), dtype=np.float32)
    for dy in range(FS):
        for u in range(20):
            for xo in range(16):
                dx = u - xo
                if 0 <= dx < FS:
                    wm[6 * u:6 * u + 6, dy, 16 * xo:16 * xo + 16] = Wd[dy, dx]
    wm[KONE, 0, :] = np.tile(np.asarray(bias1, np.float32), 16)
    return np.ascontiguousarray(wm.reshape(KDIM, FS * NMOV))


def _split_excess_waits(nc, max_waits=1):
    """This image's walrus rejects instructions carrying more than one sem
    wait ("Too many sync wait commands" in setupSyncWait). Tile freely
    attaches several waits to one instruction. Hoist the extras onto
    nofuse NOPs inserted just before, on the same engine — identical
    semantics (all waits retired before the instruction issues)."""
    import concourse.mybir as mybir

    for f in nc.m.functions:
        for bb in f.blocks:
            new_list = []
            changed = False
            for inst in bb.instructions:
                si = inst.sync_info
                waits = list(si.on_wait) if si and si.on_wait else []
                if len(waits) > max_waits:
                    changed = True
                    for k, w in enumerate(waits[max_waits:]):
                        nop = mybir.InstNoOp(
                            name=f"{inst.name}-wsplit{k}",
                            sync_info=mybir.SyncInfo(on_wait=[w], on_update=[]),
                            bass_nofuse=True,
                            engine=inst.engine,
                        )
                        new_list.append(nop)
                    si.on_wait = waits[:max_waits]
                new_list.append(inst)
            if changed:
                bb.instructions = new_list


def _build_nc(n_imgs=IMGS_PER_CORE):
    import concourse.bass as bass
    import concourse.mybir as mybir
    from concourse.tile import TileContext

    nc = bass.Bass(trn_type="TRN2")
    x = nc.dram_tensor("x", (n_imgs, 2, KDIM, XCOLS), mybir.dt.bfloat16,
                       kind="ExternalInput")
    w = nc.dram_tensor("w", (KDIM, FS * NMOV), mybir.dt.bfloat16,
                       kind="ExternalInput")
    out = nc.dram_tensor("out", (n_imgs, HO, WO, COUT), mybir.dt.bfloat16,
                         kind="ExternalOutput")

    with TileContext(nc) as tc:
        with tc.tile_pool(name="const", bufs=1) as cpool, \
             tc.tile_pool(name="xin", bufs=3) as xpool, \
             tc.tile_pool(name="stage", bufs=6) as spool, \
             tc.tile_pool(name="ps", bufs=8, space="PSUM") as ppool:
            wt = cpool.tile([KDIM, FS * NMOV], mybir.dt.bfloat16, name="wt")
            nc.sync.dma_start(out=wt[:, :], in_=w[:, :])

            # Warm the PE's HAM clock gate (needs ~3.4us of sustained matmul
            # activity to lift the 1.2GHz cold throttle) while the first
            # input DMA is in flight. A memset dummy avoids any DMA
            # dependency; ~160 tiny matmuls bridge until real data lands
            # (PE re-throttles after ~3.4us idle, so don't stop too early).
            dummy = cpool.tile([KDIM, 128], mybir.dt.bfloat16, name="dummy")
            nc.vector.memset(dummy[:, :], 0.03125)
            wps = ppool.tile([128, 512], mybir.dt.float32, name="ps", tag="ps")
            for _ in range(120):
                nc.tensor.matmul(wps[0:128, 0:64], dummy[:, 0:128],
                                 dummy[:, 0:64], start=True, stop=True)

            for n in range(n_imgs):
                for h in range(2):
                    xts = []
                    for q in range(4):
                        xtq = xpool.tile([KDIM, HPAIR * YH], mybir.dt.bfloat16,
                                         name=f"xt{q}", tag=f"xt{q}")
                        nc.scalar.dma_start(
                            out=xtq[:, :],
                            in_=x[n, h, :, q * HPAIR * YH:(q + 1) * HPAIR * YH])
                        xts.append(xtq)
                    for b in range(2):
                        cb, m = BLOCKS[h][b]
                        stg = spool.tile([128, NPAIR * NMOV], mybir.dt.bfloat16,
                                         name="stg", tag="stg")
                        y0 = 248 * h + cb
                        for pc in range(NPAIR):
                            xt = xts[pc // HPAIR]
                            ps = ppool.tile([128, 512], mybir.dt.float32,
                                            name="ps", tag="ps")
                            base = (pc % HPAIR) * YH + cb
                            for dy in range(FS):
                                nc.tensor.matmul(
                                    ps[0:m, 0:NMOV],
                                    xt[:, base + dy: base + dy + m],
                                    wt[:, dy * NMOV:(dy + 1) * NMOV],
                                    start=(dy == 0), stop=(dy == FS - 1),
                                )
                            dst = stg[0:m, pc * NMOV:(pc + 1) * NMOV]
                            if pc % 2 == 0:
                                nc.scalar.copy(dst, ps[0:m, 0:NMOV])
                            else:
                                nc.vector.tensor_copy(dst, ps[0:m, 0:NMOV])
                            if pc == NPAIR // 2 - 1:
                                nc.sync.dma_start(
                                    out=out[n, y0:y0 + m, 0:NPAIR * 8, :]
                                        .rearrange("y x c -> y (x c)"),
                                    in_=stg[0:m, 0:NPAIR * NMOV // 2],
                                )
                        nc.sync.dma_start(
                            out=out[n, y0:y0 + m, NPAIR * 8:WO, :]
                                .rearrange("y x c -> y (x c)"),
                            in_=stg[0:m, NPAIR * NMOV // 2:OCOLS],
                        )
    _split_excess_waits(nc)
    return nc


def _prep_weights(weights3, weights4, weights4_4, weights6, bias1):
    Wd = _dense_kernel_np(np.asarray(weights3, np.float32),
                          np.asarray(weights4, np.float32),
                          np.asarray(weights4_4, np.float32),
                          np.asarray(weights6, np.float32))
    return _build_wmov(Wd, bias1).astype(ml_dtypes.bfloat16)


def _prep_inputs(inputs):
    """[BATCH, 2, KDIM, NPAIR*YH] bf16: per (image, half) a transposed,
    windowed view of the image with a ones row at partition 120."""
    xin = np.asarray(inputs, np.float32).reshape(BATCH, H, FLAT)
    xpad = np.zeros((BATCH, H, FLAT_PAD), dtype=ml_dtypes.bfloat16)
    xpad[:, :, :FLAT] = xin.astype(ml_dtypes.bfloat16)
    xT = np.ascontiguousarray(xpad.transpose(0, 2, 1))  # [B, FLAT_PAD, H]
    s0, s1, s2 = xT.strides
    win = np.lib.stride_tricks.as_strided(
        xT, shape=(BATCH, 2, KWIN, NPAIR, YH),
        strides=(s0, 248 * s2, s1, 96 * s1, s2))
    xprep = np.zeros((BATCH, 2, KDIM, NPAIR, YH), dtype=ml_dtypes.bfloat16)
    xprep[:, :, :KWIN] = win
    xprep[:, :, KONE] = np.asarray(1.0, dtype=ml_dtypes.bfloat16)
    return xprep.reshape(BATCH, 2, KDIM, XCOLS)


def run(inputs, weights3, weights4, weights4_4, weights6, bias1, trace=False):
    from concourse.bass_utils import run_bass_kernel_spmd

    if "nc" not in _CACHE:
        _CACHE["nc"] = _build_nc()
    nc = _CACHE["nc"]

    w_mov = _prep_weights(weights3, weights4, weights4_4, weights6, bias1)
    xprep = _prep_inputs(inputs)

    in_maps = [
        {"x": xprep[c * IMGS_PER_CORE:(c + 1) * IMGS_PER_CORE], "w": w_mov}
        for c in range(N_CORES)
    ]
    res = run_bass_kernel_spmd(nc, in_maps, core_ids=list(range(N_CORES)),
                               trace=trace)
    out = np.concatenate([r["out"] for r in res.results], axis=0)
    return out.astype(np.float32), res


def kernel(inputs, weights3, weights4, weights4_4, weights6, bias1):
    out, _ = run(inputs, weights3, weights4, weights4_4, weights6, bias1)
    return out


# revision 30
# speedup vs baseline: 1.0042x; 1.0042x over previous
# Trainium2 Bass kernel for the LeNet-C3 sparse-connection conv problem.
#
# Math: VALID 2D conv, input [32, 512, 512, 6] f32, dense kernel [5,5,6,16]
# (assembled from the sparse C3 connection tables), + bias -> [32, 508, 508, 16].
#
# Strategy (pure data parallel, 4 images per core x 8 cores):
#   Matmul roles are "swapped" vs the obvious im2col mapping so the PSUM
#   output lands with partition = y (image row), making the output store a
#   contiguous row-block DMA:
#     psum[y, (xo,co)] += xprep[:, y+dy window].T @ W_mov[dy][:, 256]
#   - stationary (lhsT) = transposed image slice [128, M=128]: partitions are
#     the 20-pixel x 6-channel flat input window of a 16-output-pixel "pair
#     group", plus a constant ones row (row 120) that folds the bias in and
#     zero pad to 128 (128 weights enable fast-weight-load, which halves
#     LDWEIGHTS and takes MM issue spacing from 131ns to the 109ns fill
#     floor). The dy shift is a free-dim column offset (no data movement).
#   - moving (rhs) = W_mov[dy] [128, 256]: row 6u+c, col 16xo+co holds
#     Wd[dy, u-xo, c, co] (zero outside 0<=u-xo<5); row 120 of dy=0 = bias.
#   - 5 accumulating matmuls per (y-block, pair), then ScalarE/VectorE
#     (alternating) copy PSUM -> bf16 staging tile laid out exactly like the
#     DRAM output row block; two contiguous ~1MB stores per row block.
#   Engine/queue split: input loads on the scalar HWDGE ring, output stores
#   on the sync HWDGE ring, so a store's wait-for-drains never head-of-line
#   blocks input prefetch (and vice versa). ~120 dummy matmuls at kernel
#   start warm the PE HAM clock gate during the first input DMA.
#   Host side pre-transposes the input into [n, half, 128, 32*264] bf16 so
#   every device DMA is large and per-partition contiguous (a strided-AP
#   rearrange DMA is packet-flood bound: ~11ms/image on the first attempt).
#   Outputs are written bf16 and upcast to f32 on host (rel err ~3.5e-3,
#   budget 2e-2).
#
#   Measured (neuron-profile, core 0 of the real 8-core program): ~301us per
#   core for 4 images; MM stream at 109ns/MM spacing = PE fill floor for
#   this dataflow (163,840 moving columns/image @ 2.4GHz = 68.3us/image).

import numpy as np
import ml_dtypes

BATCH, H, W, CIN, COUT, FS = 32, 512, 512, 6, 16, 5
N_CORES = 8
IMGS_PER_CORE = BATCH // N_CORES  # 4
HO = WO = H - FS + 1  # 508
FLAT = W * CIN  # 3072
FLAT_PAD = 3104  # >= 96*31 + 121
NPAIR = 32          # pair groups of 16 output pixels
KWIN = 120          # 20 x-positions * 6 channels
KONE = 120          # ones row for bias
KDIM = 128          # padded to 128 so LDWEIGHTS gets fast-weight-load
NMOV = 256          # 16 out pixels * 16 out channels
YH = 264            # y extent per half: covers 2 y-blocks + 4 filter taps
XCOLS = NPAIR * YH  # 8448 free columns per (image, half) input tile
HPAIR = NPAIR // 4  # pairs per input-load chunk
OCOLS = WO * COUT   # 8128 valid output columns per row
# (half, block) -> (cb column offset within a pair's 264 cols, M rows).
# The last block re-computes rows 380-383 (overlap with block 2) so every
# store is a full 128 partitions — 124-partition DMAs only get a 4-way
# SDMA-engine split (78 GB/s vs 400).
BLOCKS = {0: ((0, 128), (128, 128)), 1: ((8, 128), (132, 128))}

_CACHE = {}


def _dense_kernel_np(weights3, weights4, weights4_4, weights6):
    """Numpy port of reference._dense_kernel: [5,5,6,16] dense conv kernel."""
    f = weights3.shape[0]
    Wd = np.zeros((f, f, CIN, COUT), dtype=np.float32)
    for i in range(6):
        for m in range(3):
            Wd[:, :, (i + m) % 6, i] = weights3[:, :, m, i]
    for k in range(6):
        for m in range(4):
            Wd[:, :, (k + m) % 6, 6 + k] = weights4[:, :, m, k]
    for k in range(3):
        for m, off in enumerate((0, 1, 3, 4)):
            Wd[:, :, (k + off) % 6, 12 + k] = weights4_4[:, :, m, k]
    Wd[:, :, :, 15] = weights6[:, :, :, 0]
    return Wd


def _build_wmov(Wd, bias1):
    """[KDIM, FS*NMOV]: col dy*256 + 16*xo + co, row 6u+c -> Wd[dy,u-xo,c,co];
    row 120 of the dy=0 slab carries the bias."""
    wm = np.zeros((KDIM, FS, NMOV), dtype=np.float32)
    for dy in range(FS):
        for u in range(20):
            for xo in range(16):
                dx = u - xo
                if 0 <= dx < FS:
                    wm[6 * u:6 * u + 6, dy, 16 * xo:16 * xo + 16] = Wd[dy, dx]
    wm[KONE, 0, :] = np.tile(np.asarray(bias1, np.float32), 16)
    return np.ascontiguousarray(wm.reshape(KDIM, FS * NMOV))


def _split_excess_waits(nc, max_waits=1):
    """This image's walrus rejects instructions carrying more than one sem
    wait ("Too many sync wait commands" in setupSyncWait). Tile freely
    attaches several waits to one instruction. Hoist the extras onto
    nofuse NOPs inserted just before, on the same engine — identical
    semantics (all waits retired before the instruction issues)."""
    import concourse.mybir as mybir

    for f in nc.m.functions:
        for bb in f.blocks:
            new_list = []
            changed = False
            for inst in bb.instructions:
                si = inst.sync_info
                waits = list(si.on_wait) if si and si.on_wait else []
                if len(waits) > max_waits:
                    changed = True
                    for k, w in enumerate(waits[max_waits:]):
                        nop = mybir.InstNoOp(
                            name=f"{inst.name}-wsplit{k}",
                            sync_info=mybir.SyncInfo(on_wait=[w], on_update=[]),
                            bass_nofuse=True,
                            engine=inst.engine,
                        )
                        new_list.append(nop)
                    si.on_wait = waits[:max_waits]
                new_list.append(inst)
            if changed:
                bb.instructions = new_list


def _build_nc(n_imgs=IMGS_PER_CORE):
    import concourse.bass as bass
    import concourse.mybir as mybir
    from concourse.tile import TileContext

    nc = bass.Bass(trn_type="TRN2")
    x = nc.dram_tensor("x", (n_imgs, 2, KDIM, XCOLS), mybir.dt.bfloat16,
                       kind="ExternalInput")
    w = nc.dram_tensor("w", (KDIM, FS * NMOV), mybir.dt.bfloat16,
                       kind="ExternalInput")
    out = nc.dram_tensor("out", (n_imgs, HO, WO, COUT), mybir.dt.bfloat16,
                         kind="ExternalOutput")

    with TileContext(nc) as tc:
        with tc.tile_pool(name="const", bufs=1) as cpool, \
             tc.tile_pool(name="xin", bufs=3) as xpool, \
             tc.tile_pool(name="stage", bufs=6) as spool, \
             tc.tile_pool(name="ps", bufs=8, space="PSUM") as ppool:
            wt = cpool.tile([KDIM, FS * NMOV], mybir.dt.bfloat16, name="wt")
            nc.sync.dma_start(out=wt[:, :], in_=w[:, :])

            # Warm the PE's HAM clock gate (needs ~3.4us of sustained matmul
            # activity to lift the 1.2GHz cold throttle) while the first
            # input DMA is in flight. A memset dummy avoids any DMA
            # dependency; ~160 tiny matmuls bridge until real data lands
            # (PE re-throttles after ~3.4us idle, so don't stop too early).
            dummy = cpool.tile([KDIM, 128], mybir.dt.bfloat16, name="dummy")
            nc.vector.memset(dummy[:, :], 0.03125)
            wps = ppool.tile([128, 512], mybir.dt.float32, name="ps", tag="ps")
            for _ in range(108):
                nc.tensor.matmul(wps[0:128, 0:64], dummy[:, 0:128],
                                 dummy[:, 0:64], start=True, stop=True)

            for n in range(n_imgs):
                for h in range(2):
                    xts = []
                    for q in range(4):
                        xtq = xpool.tile([KDIM, HPAIR * YH], mybir.dt.bfloat16,
                                         name=f"xt{q}", tag=f"xt{q}")
                        nc.scalar.dma_start(
                            out=xtq[:, :],
                            in_=x[n, h, :, q * HPAIR * YH:(q + 1) * HPAIR * YH])
                        xts.append(xtq)
                    for b in range(2):
                        cb, m = BLOCKS[h][b]
                        stg = spool.tile([128, NPAIR * NMOV], mybir.dt.bfloat16,
                                         name="stg", tag="stg")
                        y0 = 248 * h + cb
                        for pc in range(NPAIR):
                            xt = xts[pc // HPAIR]
                            ps = ppool.tile([128, 512], mybir.dt.float32,
                                            name="ps", tag="ps")
                            base = (pc % HPAIR) * YH + cb
                            for dy in range(FS):
                                nc.tensor.matmul(
                                    ps[0:m, 0:NMOV],
                                    xt[:, base + dy: base + dy + m],
                                    wt[:, dy * NMOV:(dy + 1) * NMOV],
                                    start=(dy == 0), stop=(dy == FS - 1),
                                )
                            dst = stg[0:m, pc * NMOV:(pc + 1) * NMOV]
                            if pc % 2 == 0:
                                nc.scalar.copy(dst, ps[0:m, 0:NMOV])
                            else:
                                nc.vector.tensor_copy(dst, ps[0:m, 0:NMOV])
                            # Stores in x-halves; the very last block tapers
                            # to quarters so the end-of-kernel flush is only
                            # ~0.5MB deep instead of ~1MB.
                            last_blk = (n == n_imgs - 1 and h == 1 and b == 1)
                            cuts = (15, 23, 31) if last_blk else (15, 31)
                            if pc in cuts:
                                i0 = (0 if pc == 15 else
                                      (16 if pc == 23 or not last_blk else 24))
                                x0, x1 = i0 * 16, min((pc + 1) * 16, WO)
                                nc.sync.dma_start(
                                    out=out[n, y0:y0 + m, x0:x1, :]
                                        .rearrange("y x c -> y (x c)"),
                                    in_=stg[0:m, i0 * NMOV:
                                            i0 * NMOV + (x1 - x0) * COUT],
                                )
    _split_excess_waits(nc)
    return nc


def _prep_weights(weights3, weights4, weights4_4, weights6, bias1):
    Wd = _dense_kernel_np(np.asarray(weights3, np.float32),
                          np.asarray(weights4, np.float32),
                          np.asarray(weights4_4, np.float32),
                          np.asarray(weights6, np.float32))
    return _build_wmov(Wd, bias1).astype(ml_dtypes.bfloat16)


def _prep_inputs(inputs):
    """[BATCH, 2, KDIM, NPAIR*YH] bf16: per (image, half) a transposed,
    windowed view of the image with a ones row at partition 120."""
    xin = np.asarray(inputs, np.float32).reshape(BATCH, H, FLAT)
    xpad = np.zeros((BATCH, H, FLAT_PAD), dtype=ml_dtypes.bfloat16)
    xpad[:, :, :FLAT] = xin.astype(ml_dtypes.bfloat16)
    xT = np.ascontiguousarray(xpad.transpose(0, 2, 1))  # [B, FLAT_PAD, H]
    s0, s1, s2 = xT.strides
    win = np.lib.stride_tricks.as_strided(
        xT, shape=(BATCH, 2, KWIN, NPAIR, YH),
        strides=(s0, 248 * s2, s1, 96 * s1, s2))
    xprep = np.zeros((BATCH, 2, KDIM, NPAIR, YH), dtype=ml_dtypes.bfloat16)
    xprep[:, :, :KWIN] = win
    xprep[:, :, KONE] = np.asarray(1.0, dtype=ml_dtypes.bfloat16)
    return xprep.reshape(BATCH, 2, KDIM, XCOLS)


def run(inputs, weights3, weights4, weights4_4, weights6, bias1, trace=False):
    from concourse.bass_utils import run_bass_kernel_spmd

    if "nc" not in _CACHE:
        _CACHE["nc"] = _build_nc()
    nc = _CACHE["nc"]

    w_mov = _prep_weights(weights3, weights4, weights4_4, weights6, bias1)
    xprep = _prep_inputs(inputs)

    in_maps = [
        {"x": xprep[c * IMGS_PER_CORE:(c + 1) * IMGS_PER_CORE], "w": w_mov}
        for c in range(N_CORES)
    ]
    res = run_bass_kernel_spmd(nc, in_maps, core_ids=list(range(N_CORES)),
                               trace=trace)
    out = np.concatenate([r["out"] for r in res.results], axis=0)
    return out.astype(np.float32), res


def kernel(inputs, weights3, weights4, weights4_4, weights6, bias1):
    out, _ = run(inputs, weights3, weights4, weights4_4, weights6, bias1)
    return out


# revision 31
# speedup vs baseline: 1.0063x; 1.0021x over previous
# Trainium2 Bass kernel for the LeNet-C3 sparse-connection conv problem.
#
# Math: VALID 2D conv, input [32, 512, 512, 6] f32, dense kernel [5,5,6,16]
# (assembled from the sparse C3 connection tables), + bias -> [32, 508, 508, 16].
#
# Strategy (pure data parallel, 4 images per core x 8 cores):
#   Matmul roles are "swapped" vs the obvious im2col mapping so the PSUM
#   output lands with partition = y (image row), making the output store a
#   contiguous row-block DMA:
#     psum[y, (xo,co)] += xprep[:, y+dy window].T @ W_mov[dy][:, 256]
#   - stationary (lhsT) = transposed image slice [128, M=128]: partitions are
#     the 20-pixel x 6-channel flat input window of a 16-output-pixel "pair
#     group", plus a constant ones row (row 120) that folds the bias in and
#     zero pad to 128 (128 weights enable fast-weight-load, which halves
#     LDWEIGHTS and takes MM issue spacing from 131ns to the 109ns fill
#     floor). The dy shift is a free-dim column offset (no data movement).
#   - moving (rhs) = W_mov[dy] [128, 256]: row 6u+c, col 16xo+co holds
#     Wd[dy, u-xo, c, co] (zero outside 0<=u-xo<5); row 120 of dy=0 = bias.
#   - 5 accumulating matmuls per (y-block, pair), then ScalarE/VectorE
#     (alternating) copy PSUM -> bf16 staging tile laid out exactly like the
#     DRAM output row block; two contiguous ~1MB stores per row block.
#   Engine/queue split: input loads on the scalar HWDGE ring, output stores
#   on the sync HWDGE ring, so a store's wait-for-drains never head-of-line
#   blocks input prefetch (and vice versa). ~120 dummy matmuls at kernel
#   start warm the PE HAM clock gate during the first input DMA.
#   Host side pre-transposes the input into [n, half, 128, 32*264] bf16 so
#   every device DMA is large and per-partition contiguous (a strided-AP
#   rearrange DMA is packet-flood bound: ~11ms/image on the first attempt).
#   Outputs are written bf16 and upcast to f32 on host (rel err ~3.5e-3,
#   budget 2e-2).
#
#   Measured (neuron-profile, core 0 of the real 8-core program): ~301us per
#   core for 4 images; MM stream at 109ns/MM spacing = PE fill floor for
#   this dataflow (163,840 moving columns/image @ 2.4GHz = 68.3us/image).

import numpy as np
import ml_dtypes

BATCH, H, W, CIN, COUT, FS = 32, 512, 512, 6, 16, 5
N_CORES = 8
IMGS_PER_CORE = BATCH // N_CORES  # 4
HO = WO = H - FS + 1  # 508
FLAT = W * CIN  # 3072
FLAT_PAD = 3104  # >= 96*31 + 121
NPAIR = 32          # pair groups of 16 output pixels
KWIN = 120          # 20 x-positions * 6 channels
KONE = 120          # ones row for bias
KDIM = 128          # padded to 128 so LDWEIGHTS gets fast-weight-load
NMOV = 256          # 16 out pixels * 16 out channels
YH = 264            # y extent per half: covers 2 y-blocks + 4 filter taps
XCOLS = NPAIR * YH  # 8448 free columns per (image, half) input tile
HPAIR = NPAIR // 4  # pairs per input-load chunk
OCOLS = WO * COUT   # 8128 valid output columns per row
# (half, block) -> (cb column offset within a pair's 264 cols, M rows).
# The last block re-computes rows 380-383 (overlap with block 2) so every
# store is a full 128 partitions — 124-partition DMAs only get a 4-way
# SDMA-engine split (78 GB/s vs 400).
BLOCKS = {0: ((0, 128), (128, 128)), 1: ((8, 128), (132, 128))}

_CACHE = {}


def _dense_kernel_np(weights3, weights4, weights4_4, weights6):
    """Numpy port of reference._dense_kernel: [5,5,6,16] dense conv kernel."""
    f = weights3.shape[0]
    Wd = np.zeros((f, f, CIN, COUT), dtype=np.float32)
    for i in range(6):
        for m in range(3):
            Wd[:, :, (i + m) % 6, i] = weights3[:, :, m, i]
    for k in range(6):
        for m in range(4):
            Wd[:, :, (k + m) % 6, 6 + k] = weights4[:, :, m, k]
    for k in range(3):
        for m, off in enumerate((0, 1, 3, 4)):
            Wd[:, :, (k + off) % 6, 12 + k] = weights4_4[:, :, m, k]
    Wd[:, :, :, 15] = weights6[:, :, :, 0]
    return Wd


def _build_wmov(Wd, bias1):
    """[KDIM, FS*NMOV]: col dy*256 + 16*xo + co, row 6u+c -> Wd[dy,u-xo,c,co];
    row 120 of the dy=0 slab carries the bias."""
    wm = np.zeros((KDIM, FS, NMOV), dtype=np.float32)
    for dy in range(FS):
        for u in range(20):
            for xo in range(16):
                dx = u - xo
                if 0 <= dx < FS:
                    wm[6 * u:6 * u + 6, dy, 16 * xo:16 * xo + 16] = Wd[dy, dx]
    wm[KONE, 0, :] = np.tile(np.asarray(bias1, np.float32), 16)
    return np.ascontiguousarray(wm.reshape(KDIM, FS * NMOV))


def _split_excess_waits(nc, max_waits=1):
    """This image's walrus rejects instructions carrying more than one sem
    wait ("Too many sync wait commands" in setupSyncWait). Tile freely
    attaches several waits to one instruction. Hoist the extras onto
    nofuse NOPs inserted just before, on the same engine — identical
    semantics (all waits retired before the instruction issues)."""
    import concourse.mybir as mybir

    for f in nc.m.functions:
        for bb in f.blocks:
            new_list = []
            changed = False
            for inst in bb.instructions:
                si = inst.sync_info
                waits = list(si.on_wait) if si and si.on_wait else []
                if len(waits) > max_waits:
                    changed = True
                    for k, w in enumerate(waits[max_waits:]):
                        nop = mybir.InstNoOp(
                            name=f"{inst.name}-wsplit{k}",
                            sync_info=mybir.SyncInfo(on_wait=[w], on_update=[]),
                            bass_nofuse=True,
                            engine=inst.engine,
                        )
                        new_list.append(nop)
                    si.on_wait = waits[:max_waits]
                new_list.append(inst)
            if changed:
                bb.instructions = new_list


def _build_nc(n_imgs=IMGS_PER_CORE):
    import concourse.bass as bass
    import concourse.mybir as mybir
    from concourse.tile import TileContext

    nc = bass.Bass(trn_type="TRN2")
    x = nc.dram_tensor("x", (n_imgs, 2, KDIM, XCOLS), mybir.dt.bfloat16,
                       kind="ExternalInput")
    w = nc.dram_tensor("w", (KDIM, FS * NMOV), mybir.dt.bfloat16,
                       kind="ExternalInput")
    out = nc.dram_tensor("out", (n_imgs, HO, WO, COUT), mybir.dt.bfloat16,
                         kind="ExternalOutput")

    with TileContext(nc) as tc:
        with tc.tile_pool(name="const", bufs=1) as cpool, \
             tc.tile_pool(name="xin", bufs=3) as xpool, \
             tc.tile_pool(name="stage", bufs=6) as spool, \
             tc.tile_pool(name="ps", bufs=8, space="PSUM") as ppool:
            wt = cpool.tile([KDIM, FS * NMOV], mybir.dt.bfloat16, name="wt")
            nc.sync.dma_start(out=wt[:, :], in_=w[:, :])

            # Warm the PE's HAM clock gate (needs ~3.4us of sustained matmul
            # activity to lift the 1.2GHz cold throttle) while the first
            # input DMA is in flight. A memset dummy avoids any DMA
            # dependency; ~160 tiny matmuls bridge until real data lands
            # (PE re-throttles after ~3.4us idle, so don't stop too early).
            dummy = cpool.tile([KDIM, 128], mybir.dt.bfloat16, name="dummy")
            nc.vector.memset(dummy[:, :], 0.03125)
            wps = ppool.tile([128, 512], mybir.dt.float32, name="ps", tag="ps")
            for _ in range(80):
                nc.tensor.matmul(wps[0:128, 0:64], dummy[:, 0:128],
                                 dummy[:, 0:64], start=True, stop=True)

            for n in range(n_imgs):
                for h in range(2):
                    xts = []
                    for q in range(4):
                        xtq = xpool.tile([KDIM, HPAIR * YH], mybir.dt.bfloat16,
                                         name=f"xt{q}", tag=f"xt{q}")
                        nc.scalar.dma_start(
                            out=xtq[:, :],
                            in_=x[n, h, :, q * HPAIR * YH:(q + 1) * HPAIR * YH])
                        xts.append(xtq)
                    for b in range(2):
                        cb, m = BLOCKS[h][b]
                        stg = spool.tile([128, NPAIR * NMOV], mybir.dt.bfloat16,
                                         name="stg", tag="stg")
                        y0 = 248 * h + cb
                        for pc in range(NPAIR):
                            xt = xts[pc // HPAIR]
                            ps = ppool.tile([128, 512], mybir.dt.float32,
                                            name="ps", tag="ps")
                            base = (pc % HPAIR) * YH + cb
                            for dy in range(FS):
                                nc.tensor.matmul(
                                    ps[0:m, 0:NMOV],
                                    xt[:, base + dy: base + dy + m],
                                    wt[:, dy * NMOV:(dy + 1) * NMOV],
                                    start=(dy == 0), stop=(dy == FS - 1),
                                )
                            dst = stg[0:m, pc * NMOV:(pc + 1) * NMOV]
                            if pc % 2 == 0:
                                nc.scalar.copy(dst, ps[0:m, 0:NMOV])
                            else:
                                nc.vector.tensor_copy(dst, ps[0:m, 0:NMOV])
                            # Stores in x-halves; the very last block tapers
                            # to quarters so the end-of-kernel flush is only
                            # ~0.5MB deep instead of ~1MB.
                            last_blk = (n == n_imgs - 1 and h == 1 and b == 1)
                            cuts = (15, 23, 31) if last_blk else (15, 31)
                            if pc in cuts:
                                i0 = (0 if pc == 15 else
                                      (16 if pc == 23 or not last_blk else 24))
                                x0, x1 = i0 * 16, min((pc + 1) * 16, WO)
                                nc.sync.dma_start(
                                    out=out[n, y0:y0 + m, x0:x1, :]
                                        .rearrange("y x c -> y (x c)"),
                                    in_=stg[0:m, i0 * NMOV:
                                            i0 * NMOV + (x1 - x0) * COUT],
                                )
    _split_excess_waits(nc)
    return nc


def _prep_weights(weights3, weights4, weights4_4, weights6, bias1):
    Wd = _dense_kernel_np(np.asarray(weights3, np.float32),
                          np.asarray(weights4, np.float32),
                          np.asarray(weights4_4, np.float32),
                          np.asarray(weights6, np.float32))
    return _build_wmov(Wd, bias1).astype(ml_dtypes.bfloat16)


def _prep_inputs(inputs):
    """[BATCH, 2, KDIM, NPAIR*YH] bf16: per (image, half) a transposed,
    windowed view of the image with a ones row at partition 120."""
    xin = np.asarray(inputs, np.float32).reshape(BATCH, H, FLAT)
    xpad = np.zeros((BATCH, H, FLAT_PAD), dtype=ml_dtypes.bfloat16)
    xpad[:, :, :FLAT] = xin.astype(ml_dtypes.bfloat16)
    xT = np.ascontiguousarray(xpad.transpose(0, 2, 1))  # [B, FLAT_PAD, H]
    s0, s1, s2 = xT.strides
    win = np.lib.stride_tricks.as_strided(
        xT, shape=(BATCH, 2, KWIN, NPAIR, YH),
        strides=(s0, 248 * s2, s1, 96 * s1, s2))
    xprep = np.zeros((BATCH, 2, KDIM, NPAIR, YH), dtype=ml_dtypes.bfloat16)
    xprep[:, :, :KWIN] = win
    xprep[:, :, KONE] = np.asarray(1.0, dtype=ml_dtypes.bfloat16)
    return xprep.reshape(BATCH, 2, KDIM, XCOLS)


def run(inputs, weights3, weights4, weights4_4, weights6, bias1, trace=False):
    from concourse.bass_utils import run_bass_kernel_spmd

    if "nc" not in _CACHE:
        _CACHE["nc"] = _build_nc()
    nc = _CACHE["nc"]

    w_mov = _prep_weights(weights3, weights4, weights4_4, weights6, bias1)
    xprep = _prep_inputs(inputs)

    in_maps = [
        {"x": xprep[c * IMGS_PER_CORE:(c + 1) * IMGS_PER_CORE], "w": w_mov}
        for c in range(N_CORES)
    ]
    res = run_bass_kernel_spmd(nc, in_maps, core_ids=list(range(N_CORES)),
                               trace=trace)
    out = np.concatenate([r["out"] for r in res.results], axis=0)
    return out.astype(np.float32), res


def kernel(inputs, weights3, weights4, weights4_4, weights6, bias1):
    out, _ = run(inputs, weights3, weights4, weights4_4, weights6, bias1)
    return out


# revision 33
# speedup vs baseline: 1.0096x; 1.0033x over previous
# Trainium2 Bass kernel for the LeNet-C3 sparse-connection conv problem.
#
# Math: VALID 2D conv, input [32, 512, 512, 6] f32, dense kernel [5,5,6,16]
# (assembled from the sparse C3 connection tables), + bias -> [32, 508, 508, 16].
#
# Strategy (pure data parallel, 4 images per core x 8 cores):
#   Matmul roles are "swapped" vs the obvious im2col mapping so the PSUM
#   output lands with partition = y (image row), making the output store a
#   contiguous row-block DMA:
#     psum[y, (xo,co)] += xprep[:, y+dy window].T @ W_mov[dy][:, 256]
#   - stationary (lhsT) = transposed image slice [128, M=128]: partitions are
#     the 20-pixel x 6-channel flat input window of a 16-output-pixel "pair
#     group", plus a constant ones row (row 120) that folds the bias in and
#     zero pad to 128 (128 weights enable fast-weight-load, which halves
#     LDWEIGHTS and takes MM issue spacing from 131ns to the 109ns fill
#     floor). The dy shift is a free-dim column offset (no data movement).
#   - moving (rhs) = W_mov[dy] [128, 256]: row 6u+c, col 16xo+co holds
#     Wd[dy, u-xo, c, co] (zero outside 0<=u-xo<5); row 120 of dy=0 = bias.
#   - 5 accumulating matmuls per (y-block, pair), then ScalarE/VectorE
#     (alternating) copy PSUM -> bf16 staging tile laid out exactly like the
#     DRAM output row block; two contiguous ~1MB stores per row block.
#   Engine/queue split: input loads on the scalar HWDGE ring, output stores
#   on the sync HWDGE ring, so a store's wait-for-drains never head-of-line
#   blocks input prefetch (and vice versa). ~120 dummy matmuls at kernel
#   start warm the PE HAM clock gate during the first input DMA.
#   Host side pre-transposes the input into [n, half, 128, 32*264] bf16 so
#   every device DMA is large and per-partition contiguous (a strided-AP
#   rearrange DMA is packet-flood bound: ~11ms/image on the first attempt).
#   Outputs are written bf16 and upcast to f32 on host (rel err ~3.5e-3,
#   budget 2e-2).
#
#   Measured (neuron-profile, core 0 of the real 8-core program): ~298us per
#   core for 4 images; the 2560-matmul stream runs at 109ns/MM median with
#   zero gaps >160ns = the PE fill floor for this dataflow (163,840 moving
#   columns/image @ 2.4GHz = 68.3us/image). The 5-pass structure is minimal:
#   601 contraction rows per (pair, y-block) > 4x128, and K-packing across
#   filter rows would need ~4x input traffic (partition-stacked copies),
#   which is DMA-bound-worse. fp8/DoubleRow fails the 2e-2 error budget.

import numpy as np
import ml_dtypes

BATCH, H, W, CIN, COUT, FS = 32, 512, 512, 6, 16, 5
N_CORES = 8
IMGS_PER_CORE = BATCH // N_CORES  # 4
HO = WO = H - FS + 1  # 508
FLAT = W * CIN  # 3072
FLAT_PAD = 3104  # >= 96*31 + 121
NPAIR = 32          # pair groups of 16 output pixels
KWIN = 120          # 20 x-positions * 6 channels
KONE = 120          # ones row for bias
KDIM = 128          # padded to 128 so LDWEIGHTS gets fast-weight-load
NMOV = 256          # 16 out pixels * 16 out channels
YH = 264            # y extent per half: covers 2 y-blocks + 4 filter taps
XCOLS = NPAIR * YH  # 8448 free columns per (image, half) input tile
HPAIR = NPAIR // 4  # pairs per input-load chunk
OCOLS = WO * COUT   # 8128 valid output columns per row
# (half, block) -> (cb column offset within a pair's 264 cols, M rows).
# The last block re-computes rows 380-383 (overlap with block 2) so every
# store is a full 128 partitions — 124-partition DMAs only get a 4-way
# SDMA-engine split (78 GB/s vs 400).
BLOCKS = {0: ((0, 128), (128, 128)), 1: ((8, 128), (132, 128))}

_CACHE = {}


def _dense_kernel_np(weights3, weights4, weights4_4, weights6):
    """Numpy port of reference._dense_kernel: [5,5,6,16] dense conv kernel."""
    f = weights3.shape[0]
    Wd = np.zeros((f, f, CIN, COUT), dtype=np.float32)
    for i in range(6):
        for m in range(3):
            Wd[:, :, (i + m) % 6, i] = weights3[:, :, m, i]
    for k in range(6):
        for m in range(4):
            Wd[:, :, (k + m) % 6, 6 + k] = weights4[:, :, m, k]
    for k in range(3):
        for m, off in enumerate((0, 1, 3, 4)):
            Wd[:, :, (k + off) % 6, 12 + k] = weights4_4[:, :, m, k]
    Wd[:, :, :, 15] = weights6[:, :, :, 0]
    return Wd


def _build_wmov(Wd, bias1):
    """[KDIM, FS*NMOV]: col dy*256 + 16*xo + co, row 6u+c -> Wd[dy,u-xo,c,co];
    row 120 of the dy=0 slab carries the bias."""
    wm = np.zeros((KDIM, FS, NMOV), dtype=np.float32)
    for dy in range(FS):
        for u in range(20):
            for xo in range(16):
                dx = u - xo
                if 0 <= dx < FS:
                    wm[6 * u:6 * u + 6, dy, 16 * xo:16 * xo + 16] = Wd[dy, dx]
    wm[KONE, 0, :] = np.tile(np.asarray(bias1, np.float32), 16)
    return np.ascontiguousarray(wm.reshape(KDIM, FS * NMOV))


def _split_excess_waits(nc, max_waits=1):
    """This image's walrus rejects instructions carrying more than one sem
    wait ("Too many sync wait commands" in setupSyncWait). Tile freely
    attaches several waits to one instruction. Hoist the extras onto
    nofuse NOPs inserted just before, on the same engine — identical
    semantics (all waits retired before the instruction issues)."""
    import concourse.mybir as mybir

    for f in nc.m.functions:
        for bb in f.blocks:
            new_list = []
            changed = False
            for inst in bb.instructions:
                si = inst.sync_info
                waits = list(si.on_wait) if si and si.on_wait else []
                if len(waits) > max_waits:
                    changed = True
                    for k, w in enumerate(waits[max_waits:]):
                        nop = mybir.InstNoOp(
                            name=f"{inst.name}-wsplit{k}",
                            sync_info=mybir.SyncInfo(on_wait=[w], on_update=[]),
                            bass_nofuse=True,
                            engine=inst.engine,
                        )
                        new_list.append(nop)
                    si.on_wait = waits[:max_waits]
                new_list.append(inst)
            if changed:
                bb.instructions = new_list


def _build_nc(n_imgs=IMGS_PER_CORE):
    import concourse.bass as bass
    import concourse.mybir as mybir
    from concourse.tile import TileContext

    nc = bass.Bass(trn_type="TRN2")
    x = nc.dram_tensor("x", (n_imgs, 2, KDIM, XCOLS), mybir.dt.bfloat16,
                       kind="ExternalInput")
    w = nc.dram_tensor("w", (KDIM, FS * NMOV), mybir.dt.bfloat16,
                       kind="ExternalInput")
    out = nc.dram_tensor("out", (n_imgs, HO, WO, COUT), mybir.dt.bfloat16,
                         kind="ExternalOutput")

    with TileContext(nc) as tc:
        with tc.tile_pool(name="const", bufs=1) as cpool, \
             tc.tile_pool(name="xin", bufs=3) as xpool, \
             tc.tile_pool(name="stage", bufs=6) as spool, \
             tc.tile_pool(name="ps", bufs=8, space="PSUM") as ppool:
            wt = cpool.tile([KDIM, FS * NMOV], mybir.dt.bfloat16, name="wt")
            nc.sync.dma_start(out=wt[:, :], in_=w[:, :])

            # Warm the PE's HAM clock gate (needs ~3.4us of sustained matmul
            # activity to lift the 1.2GHz cold throttle) while the first
            # input DMA is in flight. A memset dummy avoids any DMA
            # dependency; ~160 tiny matmuls bridge until real data lands
            # (PE re-throttles after ~3.4us idle, so don't stop too early).
            dummy = cpool.tile([KDIM, 128], mybir.dt.bfloat16, name="dummy")
            nc.vector.memset(dummy[:, :], 0.03125)
            wps = ppool.tile([128, 512], mybir.dt.float32, name="ps", tag="ps")
            for _ in range(80):
                nc.tensor.matmul(wps[0:128, 0:64], dummy[:, 0:128],
                                 dummy[:, 0:64], start=True, stop=True)

            for n in range(n_imgs):
                for h in range(2):
                    xts = []
                    for q in range(4):
                        xtq = xpool.tile([KDIM, HPAIR * YH], mybir.dt.bfloat16,
                                         name=f"xt{q}", tag=f"xt{q}")
                        nc.scalar.dma_start(
                            out=xtq[:, :],
                            in_=x[n, h, :, q * HPAIR * YH:(q + 1) * HPAIR * YH])
                        xts.append(xtq)
                    for b in range(2):
                        cb, m = BLOCKS[h][b]
                        stg = spool.tile([128, NPAIR * NMOV], mybir.dt.bfloat16,
                                         name="stg", tag="stg")
                        y0 = 248 * h + cb
                        for pc in range(NPAIR):
                            xt = xts[pc // HPAIR]
                            ps = ppool.tile([128, 512], mybir.dt.float32,
                                            name="ps", tag="ps")
                            base = (pc % HPAIR) * YH + cb
                            for dy in range(FS):
                                nc.tensor.matmul(
                                    ps[0:m, 0:NMOV],
                                    xt[:, base + dy: base + dy + m],
                                    wt[:, dy * NMOV:(dy + 1) * NMOV],
                                    start=(dy == 0), stop=(dy == FS - 1),
                                )
                            dst = stg[0:m, pc * NMOV:(pc + 1) * NMOV]
                            if pc % 2 == 0:
                                nc.scalar.copy(dst, ps[0:m, 0:NMOV])
                            else:
                                nc.vector.tensor_copy(dst, ps[0:m, 0:NMOV])
                            # Stores in x-halves; the very last block tapers
                            # (8+8+4+4 pairs) so the end-of-kernel flush is
                            # only ~0.26MB deep instead of ~1MB.
                            last_blk = (n == n_imgs - 1 and h == 1 and b == 1)
                            cuts = ({15: 0, 23: 16, 27: 24, 31: 28}
                                    if last_blk else {15: 0, 31: 16})
                            if pc in cuts:
                                i0 = cuts[pc]
                                x0, x1 = i0 * 16, min((pc + 1) * 16, WO)
                                nc.sync.dma_start(
                                    out=out[n, y0:y0 + m, x0:x1, :]
                                        .rearrange("y x c -> y (x c)"),
                                    in_=stg[0:m, i0 * NMOV:
                                            i0 * NMOV + (x1 - x0) * COUT],
                                )
    _split_excess_waits(nc)
    return nc


def _prep_weights(weights3, weights4, weights4_4, weights6, bias1):
    Wd = _dense_kernel_np(np.asarray(weights3, np.float32),
                          np.asarray(weights4, np.float32),
                          np.asarray(weights4_4, np.float32),
                          np.asarray(weights6, np.float32))
    return _build_wmov(Wd, bias1).astype(ml_dtypes.bfloat16)


def _prep_inputs(inputs):
    """[BATCH, 2, KDIM, NPAIR*YH] bf16: per (image, half) a transposed,
    windowed view of the image with a ones row at partition 120."""
    xin = np.asarray(inputs, np.float32).reshape(BATCH, H, FLAT)
    xpad = np.zeros((BATCH, H, FLAT_PAD), dtype=ml_dtypes.bfloat16)
    xpad[:, :, :FLAT] = xin.astype(ml_dtypes.bfloat16)
    xT = np.ascontiguousarray(xpad.transpose(0, 2, 1))  # [B, FLAT_PAD, H]
    s0, s1, s2 = xT.strides
    win = np.lib.stride_tricks.as_strided(
        xT, shape=(BATCH, 2, KWIN, NPAIR, YH),
        strides=(s0, 248 * s2, s1, 96 * s1, s2))
    xprep = np.zeros((BATCH, 2, KDIM, NPAIR, YH), dtype=ml_dtypes.bfloat16)
    xprep[:, :, :KWIN] = win
    xprep[:, :, KONE] = np.asarray(1.0, dtype=ml_dtypes.bfloat16)
    return xprep.reshape(BATCH, 2, KDIM, XCOLS)


def run(inputs, weights3, weights4, weights4_4, weights6, bias1, trace=False):
    from concourse.bass_utils import run_bass_kernel_spmd

    if "nc" not in _CACHE:
        _CACHE["nc"] = _build_nc()
    nc = _CACHE["nc"]

    w_mov = _prep_weights(weights3, weights4, weights4_4, weights6, bias1)
    xprep = _prep_inputs(inputs)

    in_maps = [
        {"x": xprep[c * IMGS_PER_CORE:(c + 1) * IMGS_PER_CORE], "w": w_mov}
        for c in range(N_CORES)
    ]
    res = run_bass_kernel_spmd(nc, in_maps, core_ids=list(range(N_CORES)),
                               trace=trace)
    out = np.concatenate([r["out"] for r in res.results], axis=0)
    return out.astype(np.float32), res


def kernel(inputs, weights3, weights4, weights4_4, weights6, bias1):
    out, _ = run(inputs, weights3, weights4, weights4_4, weights6, bias1)
    return out
